# revision 6
# baseline (speedup 1.0000x reference)
"""BEVPoolV2 (segment_reduce) on 8 Trainium2 NeuronCores.

Contract: kernel(**inputs) takes FULL unsharded inputs (depth, feat,
ranks_depth, ranks_feat, maxn) and returns the FULL (1,1,200,200,64) f32
output.

Strategy (per the voxel-slab sharding):
  - 40000 output voxels x 40 points; core k owns voxels [5000k, 5000(k+1)),
    padded to 5120 = 40 blocks x 128 voxels. No cross-core accumulation.
  - Per block (128 voxels = 5120 points), SPMD on all 8 cores:
      * dma_gather feat rows (256B each) by ranks_feat -> [128, 40, 64]
        (voxel-per-partition, point-slot along free dim; the index stream is
        host-permuted so gathered row i lands at [i%128, i//128]).
      * dma_gather 256B depth rows at ranks_depth//64 -> [128, 40, 64]; the
        exact scalar is selected on the Vector engine with a one-hot
        (iota == ranks_depth%64) mask + X-axis reduce.
      * product feat * depth, then segmented sum over the 40 slots via a
        strided tensor_reduce -> [128 voxels, 64]; DMA to HBM.
  - Gathers are 1024 descriptors per call (2048 wedges the exec unit;
    1024 verified stable), round-robined over 4 SWDGE queues, <=2 calls
    in flight per queue: ~0.8-1.0 ns/descriptor vs 2.3 at 512/call.
  - Engine split keeps the gather pipeline streaming across blocks:
    SP(sync) does ONLY idx prefetch (8-deep slots, gated on gather-queue
    sems), Activation does the output DMAs (gated on s_dve) -- an
    in-order SP doing both would stall idx prefetch behind the previous
    block's DVE completion and drain the pipeline every block.
"""
import sys
sys.path.insert(0, '/opt/trn_rl_repo')
import os
import numpy as np
import concourse.bass as bass
import concourse.bacc as bacc
from concourse import mybir
from contextlib import ExitStack

P = 128
C = 64
MAXN = 40
V_TOT = 40000            # 1*200*200 output voxels
N_CORES = 8
NV_CORE = V_TOT // N_CORES   # 5000
NB = 40                      # blocks per core (5120 padded voxels)
NVP = NB * P
PTS_BLK = P * MAXN           # 5120 points per block
DEPTH_LEN = 498432           # 1*6*118*16*44
DEP_ROWS = 7792              # ceil((DEPTH_LEN+1)/64) padded
FEAT_ROWS = 4225             # 4224 + zero row
FEAT_PAD_IDX = 4224
DEP_PAD_IDX = DEPTH_LEN
IDXW = PTS_BLK // 16         # 320
GIDX = 1024                  # idxs per dma_gather call
GCH = PTS_BLK // GIDX        # 5 chunks per table per block
CALLS_BLK = 2 * GCH          # 10 gather calls per block
NQ = 4                       # SWDGE queues
FLY = int(os.environ.get("K_FLY", "4"))  # max calls in flight per queue
NS = 4                       # gf/gd block pipeline depth (buffer slots)
NJ = 8                       # idx/rdlo prefetch depth (slots)
SP_PKT = os.environ.get("K_SINGLE_PACKET", "0") == "1"
EN_DRAIN = os.environ.get("K_DRAIN", "0") == "1"
EN_DVE = os.environ.get("K_DVE", "1") == "1"
EN_GATH = os.environ.get("K_GATH", "1") == "1"

f32 = mybir.dt.float32
i16 = mybir.dt.int16


def _wrap_idx(idx, dtype=np.int16):
    """[..., N] -> [..., 128, N//16]: idx i at [i%16, i//16], replicated x8."""
    n = idx.shape[-1]
    w = idx.reshape(*idx.shape[:-1], n // 16, 16)
    w = np.swapaxes(w, -1, -2)
    w = np.broadcast_to(w[..., None, :, :], (*idx.shape[:-1], 8, 16, n // 16))
    return np.ascontiguousarray(
        w.reshape(*idx.shape[:-1], 128, n // 16)).astype(dtype)


def _host_prep(depth, feat, ranks_depth, ranks_feat):
    depth_flat = np.asarray(depth, np.float32).reshape(-1)
    dep_tab = np.zeros((DEP_ROWS, C), np.float32)
    dep_tab.reshape(-1)[:DEPTH_LEN] = depth_flat
    feat_tab = np.zeros((FEAT_ROWS, C), np.float32)
    feat_tab[:FEAT_ROWS - 1] = np.asarray(feat, np.float32).reshape(-1, C)
    iota = np.broadcast_to(np.arange(C, dtype=np.float32), (P, C)).copy()

    rd = np.asarray(ranks_depth, np.int64).reshape(V_TOT, MAXN)
    rf = np.asarray(ranks_feat, np.int64).reshape(V_TOT, MAXN)

    in_maps = []
    for k in range(N_CORES):
        rd_c = np.full((NVP, MAXN), DEP_PAD_IDX, np.int64)
        rf_c = np.full((NVP, MAXN), FEAT_PAD_IDX, np.int64)
        rd_c[:NV_CORE] = rd[k * NV_CORE:(k + 1) * NV_CORE]
        rf_c[:NV_CORE] = rf[k * NV_CORE:(k + 1) * NV_CORE]
        rd_b = rd_c.reshape(NB, P, MAXN).transpose(0, 2, 1)   # [NB, s, p]
        rf_b = rf_c.reshape(NB, P, MAXN).transpose(0, 2, 1)
        in_maps.append({
            "dep_tab": dep_tab,
            "feat_tab": feat_tab,
            "rf_w": _wrap_idx(rf_b.reshape(NB, PTS_BLK)),
            "rdhi_w": _wrap_idx((rd_b // C).reshape(NB, PTS_BLK)),
            "rdlo": np.ascontiguousarray(
                (rd_b % C).astype(np.float32).transpose(0, 2, 1)),
            "iota": iota,
        })
    return in_maps


def _build_kernel(nb=NB, m_rep=1):
    nc = bacc.Bacc("TRN2", debug=False, num_swdge_queues=NQ)
    dep_tab = nc.dram_tensor("dep_tab", [DEP_ROWS, C], f32, kind="ExternalInput")
    feat_tab = nc.dram_tensor("feat_tab", [FEAT_ROWS, C], f32, kind="ExternalInput")
    rf_w = nc.dram_tensor("rf_w", [nb, P, IDXW], i16, kind="ExternalInput")
    rdhi_w = nc.dram_tensor("rdhi_w", [nb, P, IDXW], i16, kind="ExternalInput")
    rdlo = nc.dram_tensor("rdlo", [nb, P, MAXN], f32, kind="ExternalInput")
    iota = nc.dram_tensor("iota", [P, C], f32, kind="ExternalInput")
    out = nc.dram_tensor("out", [nb, P, C], f32, kind="ExternalOutput")

    T = nb * m_rep
    # static per-queue schedule: global call g = CALLS_BLK*t + i runs on
    # queue g % NQ.  cum_q[t][q] = # calls on q among blocks 0..t.
    cum_q = []
    cnt = [0] * NQ
    for t in range(T):
        for i in range(CALLS_BLK):
            cnt[(CALLS_BLK * t + i) % NQ] += 1
        cum_q.append(list(cnt))

    with ExitStack() as st:
        e = st.enter_context
        rf_sb = e(nc.sbuf_tensor("rf_sb", [P, NJ, IDXW], i16))
        rdhi_sb = e(nc.sbuf_tensor("rdhi_sb", [P, NJ, IDXW], i16))
        rdlo_sb = e(nc.sbuf_tensor("rdlo_sb", [P, NJ, MAXN], f32))
        gf_sb = e(nc.sbuf_tensor("gf_sb", [P, NS, MAXN, C], f32))
        gd_sb = e(nc.sbuf_tensor("gd_sb", [P, NS, MAXN, C], f32))
        mask_sb = e(nc.sbuf_tensor("mask_sb", [P, MAXN, C], f32))
        d_sb = e(nc.sbuf_tensor("d_sb", [P, MAXN], f32))
        o_sb = e(nc.sbuf_tensor("o_sb", [P, NS, C], f32))
        iota_sb = e(nc.sbuf_tensor("iota_sb", [P, C], f32))
        s_cst = e(nc.semaphore("s_cst"))
        s_idx = [e(nc.semaphore(f"s_idx{j}")) for j in range(NJ)]
        s_q = [e(nc.semaphore(f"s_q{q}")) for q in range(NQ)]
        s_out = [e(nc.semaphore(f"s_out{j}")) for j in range(NS)]
        s_dve = e(nc.semaphore("s_dve"))
        blk = e(nc.Block())

        @blk.sync
        def _(sync):
            # idx prefetch ONLY -- runs NJ blocks ahead of the gathers.
            sync.dma_start(iota_sb[:], iota[:]).then_inc(s_cst, 16)
            for t in range(T):
                b, j2 = t % nb, t % NJ
                if t >= NJ:
                    # rf/rdhi slot reuse: gathers of block t-NJ consumed them
                    if EN_GATH:
                        for q in range(NQ):
                            sync.wait_ge(s_q[q], 16 * cum_q[t - NJ][q])
                    # rdlo slot reuse: DVE consumed it
                    sync.wait_ge(s_dve, t - NJ + 1)
                sync.dma_start(rf_sb[:, j2], rf_w[b]).then_inc(s_idx[j2], 16)
                sync.dma_start(rdhi_sb[:, j2], rdhi_w[b]).then_inc(s_idx[j2], 16)
                sync.dma_start(rdlo_sb[:, j2], rdlo[b]).then_inc(s_idx[j2], 16)

        @blk.scalar
        def _(scalar):
            # output DMAs on the Activation engine's HWDGE queue
            for t in range(T):
                b, j, k = t % nb, t % NS, t // NS
                scalar.wait_ge(s_dve, t + 1)
                if t >= NS:
                    scalar.wait_ge(s_out[j], 16 * k)
                scalar.dma_start(out[b], o_sb[:, j]).then_inc(s_out[j], 16)

        @blk.gpsimd
        def _(gpsimd):
            issued = [0] * NQ
            for t in range(T):
                b, j, j2, k = t % nb, t % NS, t % NJ, t // NJ
                gpsimd.wait_ge(s_idx[j2], 48 * (k + 1))
                if t >= NS:
                    # gf/gd slot reuse: vector consumed block t-NS
                    gpsimd.wait_ge(s_dve, t - NS + 1)
                wl = GIDX // 16   # 64 idx columns per call
                sl = GIDX // P    # 8 slot-rows per call
                nreg = 904 if b == nb - 1 else GIDX
                for i in range(CALLS_BLK):
                    if not EN_GATH:
                        break
                    q = (CALLS_BLK * t + i) % NQ
                    issued[q] += 1
                    if issued[q] > FLY:
                        gpsimd.wait_ge(s_q[q], 16 * (issued[q] - FLY))
                    c = i // 2
                    if i % 2 == 0:
                        gpsimd.dma_gather(
                            gf_sb[:, j, sl*c:sl*(c+1)], feat_tab[:],
                            rf_sb[:, j2, wl*c:wl*(c+1)], GIDX, nreg, C,
                            queue_num=q, single_packet=SP_PKT,
                        ).then_inc(s_q[q], 16)
                    else:
                        gpsimd.dma_gather(
                            gd_sb[:, j, sl*c:sl*(c+1)], dep_tab[:],
                            rdhi_sb[:, j2, wl*c:wl*(c+1)], GIDX, nreg, C,
                            queue_num=q, single_packet=SP_PKT,
                        ).then_inc(s_q[q], 16)

        @blk.vector
        def _(vector):
            for t in range(T):
                j, j2, k = t % NS, t % NJ, t // NJ
                if t == 0:
                    vector.wait_ge(s_cst, 16)
                vector.wait_ge(s_idx[j2], 48 * (k + 1))
                if not EN_DVE:
                    if EN_GATH:
                        for q in range(NQ):
                            vector.wait_ge(s_q[q], 16 * cum_q[t][q])
                    if t >= NS:
                        vector.wait_ge(s_out[j], 16 * (t // NS))
                    vector.reduce_sum(out=d_sb[:, :1], in_=rdlo_sb[:, j2, :2],
                                      axis=mybir.AxisListType.X
                                      ).then_inc(s_dve, 1)
                    continue
                vector.tensor_tensor(
                    out=mask_sb[:],
                    in0=iota_sb[:][:, None, :].to_broadcast([P, MAXN, C]),
                    in1=rdlo_sb[:, j2][:, :, None].to_broadcast([P, MAXN, C]),
                    op=mybir.AluOpType.is_equal)
                if EN_GATH:
                    for q in range(NQ):
                        vector.wait_ge(s_q[q], 16 * cum_q[t][q])
                if EN_DRAIN:
                    vector.drain()
                vector.tensor_tensor(out=mask_sb[:], in0=mask_sb[:],
                                     in1=gd_sb[:, j], op=mybir.AluOpType.mult)
                if EN_DRAIN:
                    vector.drain()
                vector.reduce_sum(out=d_sb[:], in_=mask_sb[:],
                                  axis=mybir.AxisListType.X)
                if EN_DRAIN:
                    vector.drain()
                vector.tensor_tensor(
                    out=gf_sb[:, j], in0=gf_sb[:, j],
                    in1=d_sb[:][:, :, None].to_broadcast([P, MAXN, C]),
                    op=mybir.AluOpType.mult)
                if EN_DRAIN:
                    vector.drain()
                if t >= NS:
                    vector.wait_ge(s_out[j], 16 * (t // NS))
                vector.reduce_sum(
                    out=o_sb[:, j],
                    in_=gf_sb[:, j].rearrange("p s c -> p c s"),
                    axis=mybir.AxisListType.X).then_inc(s_dve, 1)

    nc.compile()
    return nc


def build_kernel(m_rep=1):
    return _build_kernel(m_rep=m_rep)


_NC_CACHE = None


def kernel(depth, feat, ranks_depth, ranks_feat, maxn):
    global _NC_CACHE
    from concourse.bass_utils import run_bass_kernel_spmd
    assert int(maxn) == MAXN
    in_maps = _host_prep(depth, feat, ranks_depth, ranks_feat)
    if _NC_CACHE is None:
        _NC_CACHE = _build_kernel()
    res = run_bass_kernel_spmd(_NC_CACHE, in_maps, core_ids=list(range(N_CORES)))
    parts = [r["out"].reshape(NVP, C)[:NV_CORE] for r in res.results]
    return np.concatenate(parts, 0).reshape(1, 1, 200, 200, C)
